# revision 1
# baseline (speedup 1.0000x reference)
"""Distributed Trainium2 kernel for nn_Attention_31104153157828.

Computation (B=16, S=2048, D=1024):
    fac1 = k @ W                     [B,S,D]
    fac2 = (q @ U)[:, None, :]       [B,1,D]
    t    = tanh(fac1 + fac2)
    s    = einsum('bsd,bse->bde', v, t)      [B,D,D]
    attn = softmax(s, axis=0)                 (softmax over BATCH)
    out  = einsum('bsd,bde->bse', v, attn)   [B,S,D]

Sharding: data-parallel over batch, 2 batches per core on 8 cores.
The batch-axis softmax needs cross-core AllReduce of max and sum(exp)
over the [D,D] logit matrix, pipelined by e-half so the AllReduces hide
under tensor-engine work:
  A(b0), A(b1); B(h0) -> AR-max(h0); B(h1) || (exp+AR-sum(h0));
  C(h0) || (AR-max(h1)+exp+AR-sum(h1)); C(h1)

Queue discipline (in-order engine queues must never hold AR-gated work
in front of compute-critical work):
  - v is pre-cast to a DRAM bf16 scratch early; all later v loads are
    plain HWDGE loads, so the GpSimd queue carries only the AR triggers
    and the softmax elementwise ops (max/sub/sum/mul), which ARE the
    AR chain.
  - Vector queue carries only PSUM evictions (B, C) and kT copies.
  - Scalar queue: tanh/exp/ln activations + softmax bounce DMAs.
Stage C runs kc-outer/m-inner over 8 persistent PSUM banks, mirroring
stage B's accumulation pattern (keeps the PE clock warm).
"""
import numpy as np
import concourse.bass as bass
import concourse.bacc as bacc
import concourse.tile as tile
import concourse.mybir as mybir
from concourse.bass_utils import run_bass_kernel_spmd

F32 = mybir.dt.float32
F32R = mybir.dt.float32r
BF16 = mybir.dt.bfloat16
AF = mybir.ActivationFunctionType

B, S, D = 16, 2048, 1024
N_CORES = 8
BL = B // N_CORES          # local batches per core = 2
M_T = S // 128             # 16 s-tiles
KC = D // 128              # 8 contraction chunks (d)
EH = 2                     # e halves of 512
ARC = 4                    # softmax chunks (pairs of d-tiles)
RG = [list(range(N_CORES))]


def build():
    nc = bacc.Bacc("TRN2", target_bir_lowering=False, debug=False,
                   num_devices=N_CORES)

    q2 = nc.dram_tensor("q2", [BL, D], F32, kind="ExternalInput")
    k2 = nc.dram_tensor("k2", [BL, S, D], F32, kind="ExternalInput")
    v2 = nc.dram_tensor("v2", [BL, S, D], F32, kind="ExternalInput")
    Wd = nc.dram_tensor("W", [D, D], F32, kind="ExternalInput")
    Ud = nc.dram_tensor("U", [D, D], F32, kind="ExternalInput")
    out2 = nc.dram_tensor("out", [BL, S, D], F32, kind="ExternalOutput")

    v_bf = nc.dram_tensor("v_bf", [BL, S, D], BF16)

    # collective bounce buffers, one set per e-half
    mx_in = [nc.dram_tensor(f"mx_in{h}", [128, KC, 512], BF16) for h in range(EH)]
    mx_out = [nc.dram_tensor(f"mx_out{h}", [128, KC, 512], BF16) for h in range(EH)]
    sm_in = [nc.dram_tensor(f"sm_in{h}", [128, KC, 512], BF16) for h in range(EH)]
    sm_out = [nc.dram_tensor(f"sm_out{h}", [128, KC, 512], BF16) for h in range(EH)]

    warm_in = nc.dram_tensor("warm_in", [128, 16], F32)
    warm_out = nc.dram_tensor("warm_out", [128, 16], F32)
    warm_out2 = nc.dram_tensor("warm_out2", [128, 16], F32)

    ident_d = nc.inline_tensor(np.eye(128, dtype=np.float32), name="ident")
    ones_d = nc.inline_tensor(np.ones((1, 128), np.float32), name="ones1")

    with tile.TileContext(nc) as tc:
        with tc.tile_pool(name="rp", bufs=1) as rp:
            ident = rp.tile([128, 128], F32, name="ident_t")
            nc.sync.dma_start(ident[:], ident_d.ap())

            # warm up the collective machinery early (first AR pays ~70us)
            wtile = rp.tile([128, 16], F32, name="wtile")
            nc.gpsimd.dma_start(wtile[:], ident_d.ap()[:, 0:16])
            nc.gpsimd.dma_start(warm_in.ap(), wtile[:])
            ar_w1 = nc.gpsimd.collective_compute(
                "AllReduce", mybir.AluOpType.max, replica_groups=RG,
                ins=[warm_in.ap().opt()], outs=[warm_out.ap().opt()])
            ar_w2 = nc.gpsimd.collective_compute(
                "AllReduce", mybir.AluOpType.add, replica_groups=RG,
                ins=[warm_out.ap().opt()], outs=[warm_out2.ap().opt()])

            # ---- t (tanh output), bf16, both batches resident ----
            tp_cm = tc.tile_pool(name="tp", bufs=1)
            tp = tp_cm.__enter__()
            t_sb = [tp.tile([128, M_T, D], BF16, name=f"t{b}") for b in range(BL)]

            # ---- stage A residents: W, fac2 (freed after A) ----
            wp_cm = tc.tile_pool(name="wp", bufs=1)
            wp = wp_cm.__enter__()
            W_r = wp.tile([128, KC, D], F32R, name="W_r")
            nc.gpsimd.dma_start(W_r[:], Wd.ap().rearrange("(kc p) e -> p kc e", p=128))
            ones_r = wp.tile([1, 128], F32R, name="ones_r")
            nc.gpsimd.dma_start(ones_r[:], ones_d.ap())

            # ---- fac2 = q @ U, per local batch -> [1, BL, D] f32r ----
            fac2 = wp.tile([1, BL, D], F32R, name="fac2")
            with (
                tc.tile_pool(name="f2", bufs=2) as f2p,
                tc.tile_pool(name="f2u", bufs=1) as f2u,
                tc.tile_pool(name="f2ps", bufs=2, space="PSUM") as f2ps,
            ):
                U_r = f2u.tile([128, KC, D], F32R, name="U_r")
                nc.gpsimd.dma_start(
                    U_r[:], Ud.ap().rearrange("(kc p) e -> p kc e", p=128))
                for b in range(BL):
                    qcol = f2p.tile([128, KC], F32R, tag="qcol", name=f"qcol{b}")
                    nc.gpsimd.dma_start(
                        qcol[:], q2.ap()[b].rearrange("(kc p) -> p kc", p=128))
                    for h in range(EH):
                        ps = f2ps.tile([1, 512], F32, tag="f2ps", name=f"f2ps{b}_{h}")
                        for kc in range(KC):
                            nc.tensor.matmul(ps[:], qcol[:, kc:kc + 1],
                                             U_r[:, kc, h * 512:(h + 1) * 512],
                                             start=(kc == 0), stop=(kc == KC - 1))
                        nc.scalar.copy(fac2[0:1, b, h * 512:(h + 1) * 512], ps[:])

            # pre-cast v to bf16 in DRAM during stage A (after the small
            # gpsimd loads so it doesn't stall stage A's weights)
            for b in range(BL):
                nc.gpsimd.dma_start(v_bf.ap()[b], v2.ap()[b])

            # ======== stage A per local batch: t = tanh(k @ W + fac2) ========
            with (
                tc.tile_pool(name="ak", bufs=3) as akp,
                tc.tile_pool(name="akt", bufs=2) as ktp,
                tc.tile_pool(name="aps", bufs=2, space="PSUM") as aps,
                tc.tile_pool(name="atp", bufs=2, space="PSUM") as tps,
            ):
                for b in range(BL):
                    for m in range(M_T):
                        kslab = akp.tile([128, D], F32, tag="kslab",
                                         name=f"kslab{b}_{m}")
                        nc.sync.dma_start(
                            kslab[:], k2.ap()[b, m * 128:(m + 1) * 128, :])
                        kT = ktp.tile([128, KC * 128], F32R, tag="kT",
                                      name=f"kT{b}_{m}")
                        for kc in range(KC):
                            ptr = tps.tile([128, 128], F32, tag="ptr",
                                           name=f"ptr{b}_{m}_{kc}")
                            nc.tensor.transpose(
                                ptr[:], kslab[:, kc * 128:(kc + 1) * 128], ident[:])
                            nc.vector.tensor_copy(kT[:, kc * 128:(kc + 1) * 128],
                                                  ptr[:])
                        psh = [aps.tile([128, 512], F32, tag=f"aps{h}",
                                        name=f"aps{b}_{m}_{h}") for h in range(EH)]
                        for kc in range(KC):
                            for h in range(EH):
                                nc.tensor.matmul(
                                    psh[h][:], kT[:, kc * 128:(kc + 1) * 128],
                                    W_r[:, kc, h * 512:(h + 1) * 512],
                                    start=(kc == 0), stop=False)
                        for h in range(EH):
                            nc.tensor.matmul(
                                psh[h][:], ones_r[:],
                                fac2[0:1, b, h * 512:(h + 1) * 512],
                                start=False, stop=True)
                            nc.scalar.activation(
                                t_sb[b][:, m, h * 512:(h + 1) * 512],
                                psh[h][:], AF.Tanh)

            wp_cm.__exit__(None, None, None)

            # ======== stages B + softmax + C, pipelined by e-half ========
            sm_cm = tc.tile_pool(name="smx", bufs=2)
            smx = sm_cm.__enter__()
            sfp_cm = tc.tile_pool(name="sfp", bufs=2)
            sfp = sfp_cm.__enter__()

            bp_cm = tc.tile_pool(name="bp", bufs=3)
            bp = bp_cm.__enter__()
            bps_cm = tc.tile_pool(name="bps", bufs=1, space="PSUM")
            bps = bps_cm.__enter__()

            def stage_b(h):
                s_h = []
                for b in range(BL):
                    psb = [bps.tile([128, 512], F32, tag=f"pb{dt}",
                                    name=f"pb{h}_{b}_{dt}") for dt in range(KC)]
                    for m in range(M_T):
                        vslab = bp.tile([128, D], BF16, tag="vslab",
                                        name=f"vslab{h}_{b}_{m}")
                        nc.sync.dma_start(
                            vslab[:], v_bf.ap()[b, m * 128:(m + 1) * 128, :])
                        for dt in range(KC):
                            nc.tensor.matmul(
                                psb[dt][:],
                                vslab[:, dt * 128:(dt + 1) * 128],
                                t_sb[b][:, m, h * 512:(h + 1) * 512],
                                start=(m == 0), stop=(m == M_T - 1))
                    s_b = sfp.tile([128, KC, 512], F32, tag=f"s{b}",
                                   name=f"s{h}_{b}")
                    for dt in range(KC):
                        last_evict = nc.vector.tensor_copy(s_b[:, dt, :],
                                                           psb[dt][:])
                    s_h.append(s_b)
                return s_h, last_evict

            def local_max(h, s_h):
                for c in range(ARC):
                    dsl = slice(2 * c, 2 * c + 2)
                    mx = smx.tile([128, 2, 512], BF16, tag="sfb", name=f"mx{h}_{c}")
                    nc.vector.tensor_max(mx[:], s_h[0][:, dsl, :],
                                         s_h[1][:, dsl, :])
                    nc.gpsimd.dma_start(mx_in[h].ap()[:, dsl, :], mx[:])

            def exp_and_sum(h, s_h, p_h):
                # after AR-max(h): subtract gmax, exp -> p bf16, local sum
                for c in range(ARC):
                    dsl = slice(2 * c, 2 * c + 2)
                    gmxb = smx.tile([128, 2, 512], BF16, tag="sfb",
                                    name=f"gmxb{h}_{c}")
                    nc.gpsimd.dma_start(gmxb[:], mx_out[h].ap()[:, dsl, :])
                    gmx = smx.tile([128, 2, 512], F32, tag="sff",
                                   name=f"gmx{h}_{c}")
                    nc.vector.tensor_copy(gmx[:], gmxb[:])
                    for b in range(BL):
                        nc.vector.tensor_sub(s_h[b][:, dsl, :],
                                             s_h[b][:, dsl, :], gmx[:])
                        nc.scalar.activation(p_h[b][:, dsl, :],
                                             s_h[b][:, dsl, :], AF.Exp)
                    sm = smx.tile([128, 2, 512], BF16, tag="sfb", name=f"sm{h}_{c}")
                    nc.vector.tensor_add(sm[:], p_h[0][:, dsl, :],
                                         p_h[1][:, dsl, :])
                    nc.gpsimd.dma_start(sm_in[h].ap()[:, dsl, :], sm[:])

            def attn_mul(h, p_h):
                # after AR-sum(h): 1/Z = exp(-ln(Z)); attn = p * rec, in place
                for c in range(ARC):
                    dsl = slice(2 * c, 2 * c + 2)
                    zz = smx.tile([128, 2, 512], BF16, tag="sfb", name=f"zz{h}_{c}")
                    nc.gpsimd.dma_start(zz[:], sm_out[h].ap()[:, dsl, :])
                    rec = smx.tile([128, 2, 512], F32, tag="sff",
                                   name=f"rec{h}_{c}")
                    nc.scalar.activation(rec[:], zz[:], AF.Ln)
                    recb = smx.tile([128, 2, 512], BF16, tag="sfb",
                                    name=f"recb{h}_{c}")
                    nc.scalar.activation(recb[:], rec[:], AF.Exp, scale=-1.0)
                    for b in range(BL):
                        nc.vector.tensor_mul(p_h[b][:, dsl, :],
                                             p_h[b][:, dsl, :], recb[:])

            # ---- pipelined execution over e-halves ----
            p_t = {}
            for h in range(EH):
                p_t[h] = [sfp.tile([128, KC, 512], BF16, tag=f"p{b}",
                                   name=f"p{h}_{b}") for b in range(BL)]

            s0, _ = stage_b(0)
            local_max(0, s0)
            ar_mx0 = nc.gpsimd.collective_compute(
                "AllReduce", mybir.AluOpType.max, replica_groups=RG,
                ins=[mx_in[0].ap().opt()], outs=[mx_out[0].ap().opt()])
            tile.add_dep_helper(ar_mx0.ins, ar_w2.ins, sync=False,
                                reason="serialize collectives")

            s1, _ = stage_b(1)  # PE busy while AR-max(h0) flies
            exp_and_sum(0, s0, p_t[0])
            ar_sm0 = nc.gpsimd.collective_compute(
                "AllReduce", mybir.AluOpType.add, replica_groups=RG,
                ins=[sm_in[0].ap().opt()], outs=[sm_out[0].ap().opt()])
            tile.add_dep_helper(ar_sm0.ins, ar_mx0.ins, sync=False,
                                reason="serialize collectives")

            local_max(1, s1)
            ar_mx1 = nc.gpsimd.collective_compute(
                "AllReduce", mybir.AluOpType.max, replica_groups=RG,
                ins=[mx_in[1].ap().opt()], outs=[mx_out[1].ap().opt()])
            tile.add_dep_helper(ar_mx1.ins, ar_sm0.ins, sync=False,
                                reason="serialize collectives")

            bps_cm.__exit__(None, None, None)
            bp_cm.__exit__(None, None, None)

            cp_cm = tc.tile_pool(name="cp", bufs=12)
            cp = cp_cm.__enter__()
            op_cm = tc.tile_pool(name="op", bufs=3)
            op = op_cm.__enter__()
            cps_cm = tc.tile_pool(name="cps", bufs=1, space="PSUM")
            cps = cps_cm.__enter__()

            def stage_c_round(h, attn_h, b, mg):
                # kc-outer / m-inner over 8 persistent PSUM banks: mirrors
                # stage B's long-accumulation pattern. v^T tiles arrive via
                # DRAM->SBUF xbar transpose, just-in-time per 8-tile round.
                if True:
                    if True:
                        vts = []
                        for j in range(8):
                            m = mg + j
                            vt = cp.tile([128, KC, 128], BF16, tag="vt",
                                         name=f"vt{h}_{b}_{m}")
                            nc.sync.dma_start(
                                vt[:], v_bf.ap()[b, m * 128:(m + 1) * 128, :],
                                transpose=True)
                            vts.append(vt)
                        pss = [cps.tile([128, 512], F32, tag=f"cps{j}",
                                        name=f"cps{h}_{b}_{mg}_{j}")
                               for j in range(8)]
                        for kc in range(KC):
                            for j in range(8):
                                nc.tensor.matmul(
                                    pss[j][:], vts[j][:, kc, :],
                                    attn_h[b][:, kc, :],
                                    start=(kc == 0), stop=(kc == KC - 1))
                        for j in range(8):
                            m = mg + j
                            ost = op.tile([128, 512], F32, tag="ost",
                                          name=f"ost{h}_{b}_{m}")
                            nc.scalar.copy(ost[:], pss[j][:])
                            nc.sync.dma_start(
                                out2.ap()[b, m * 128:(m + 1) * 128,
                                          h * 512:(h + 1) * 512], ost[:])

            attn_mul(0, p_t[0])
            stage_c_round(0, p_t[0], 0, 0)   # PE busy while AR(h1) flies
            exp_and_sum(1, s1, p_t[1])
            stage_c_round(0, p_t[0], 0, 8)
            ar_sm1 = nc.gpsimd.collective_compute(
                "AllReduce", mybir.AluOpType.add, replica_groups=RG,
                ins=[sm_in[1].ap().opt()], outs=[sm_out[1].ap().opt()])
            tile.add_dep_helper(ar_sm1.ins, ar_mx1.ins, sync=False,
                                reason="serialize collectives")
            stage_c_round(0, p_t[0], 1, 0)
            attn_mul(1, p_t[1])
            stage_c_round(0, p_t[0], 1, 8)

            for b in range(BL):
                for mg in (0, 8):
                    stage_c_round(1, p_t[1], b, mg)


            cps_cm.__exit__(None, None, None)
            op_cm.__exit__(None, None, None)
            cp_cm.__exit__(None, None, None)
            sfp_cm.__exit__(None, None, None)
            sm_cm.__exit__(None, None, None)
            tp_cm.__exit__(None, None, None)

    nc.compile()
    return nc


_NC = None


def _get_nc():
    global _NC
    if _NC is None:
        _NC = build()
    return _NC


def kernel(q, k, v, W, U):
    q = np.ascontiguousarray(np.asarray(q, dtype=np.float32))
    k = np.ascontiguousarray(np.asarray(k, dtype=np.float32))
    v = np.ascontiguousarray(np.asarray(v, dtype=np.float32))
    W = np.ascontiguousarray(np.asarray(W, dtype=np.float32))
    U = np.ascontiguousarray(np.asarray(U, dtype=np.float32))

    nc = _get_nc()
    in_maps = [
        {
            "q2": q[c * BL:(c + 1) * BL],
            "k2": k[c * BL:(c + 1) * BL],
            "v2": v[c * BL:(c + 1) * BL],
            "W": W,
            "U": U,
        }
        for c in range(N_CORES)
    ]
    res = run_bass_kernel_spmd(nc, in_maps, core_ids=list(range(N_CORES)))
    out = np.concatenate([res.results[c]["out"] for c in range(N_CORES)], axis=0)
    return out.astype(np.float32)


if __name__ == "__main__":
    rng = np.random.default_rng(0)
    q = rng.standard_normal((B, D), dtype=np.float32)
    k = rng.standard_normal((B, S, D), dtype=np.float32)
    v = rng.standard_normal((B, S, D), dtype=np.float32)
    W = (rng.standard_normal((D, D), dtype=np.float32) / np.sqrt(D)).astype(np.float32)
    U = (rng.standard_normal((D, D), dtype=np.float32) / np.sqrt(D)).astype(np.float32)
    out = kernel(q=q, k=k, v=v, W=W, U=U)
    print("out", out.shape, out.dtype, float(np.abs(out).mean()))



# revision 7
# speedup vs baseline: 1.0399x; 1.0399x over previous
"""Distributed Trainium2 kernel for nn_Attention_31104153157828.

Computation (B=16, S=2048, D=1024):
    fac1 = k @ W                     [B,S,D]
    fac2 = (q @ U)[:, None, :]       [B,1,D]
    t    = tanh(fac1 + fac2)
    s    = einsum('bsd,bse->bde', v, t)      [B,D,D]
    attn = softmax(s, axis=0)                 (softmax over BATCH)
    out  = einsum('bsd,bde->bse', v, attn)   [B,S,D]

Sharding: data-parallel over batch, 2 batches per core on 8 cores.
The batch-axis softmax needs a cross-core AllReduce of max and sum(exp)
over the [D,D] logit matrix, pipelined by e-half so the AllReduces hide
under tensor-engine work.

v2 design (vs the v1 baseline at ~890us):
  - All matmuls fp16 (same PE rate as bf16/f32r at N=512, 4x less
    rounding error than bf16). k/W/v/t/attn fp16, PSUM/logits f32.
  - k cast to a DRAM fp16 scratch; kT tiles arrive via HWDGE
    xbar-transpose loads: v1's 256 PE transposes + 256 DVE copies gone.
  - In-order engine queues are laid out so no AR-gated instruction ever
    sits in front of compute-critical work:
      PE:     fac2 MMs, A MMs, B MMs, C MMs (program order = pipeline)
      Scalar: tanh, softmax exp, mx/sm bounce STORES, C out-stores
      Vector: B evicts, h0 softmax chain, 1/Z, attn muls, C evicts
      GpSimd: casts, AR triggers + AR-gated bounce LOADS, h1 pre-sum
              softmax chain (so vector never blocks C(h0) evictions)
      Sync:   kT transposes, v slabs, vT transposes (never AR-gated)
"""
import numpy as np
import concourse.bass as bass
import concourse.bacc as bacc
import concourse.tile as tile
import concourse.mybir as mybir
from concourse.bass_utils import run_bass_kernel_spmd

F32 = mybir.dt.float32
F32R = mybir.dt.float32r
F16 = mybir.dt.float16
AF = mybir.ActivationFunctionType

B, S, D = 16, 2048, 1024
N_CORES = 8
BL = B // N_CORES          # local batches per core = 2
M_T = S // 128             # 16 s-tiles
KC = D // 128              # 8 contraction chunks (d)
EH = 2                     # e halves of 512
ARC = 4                    # softmax chunks (pairs of d-tiles)
MG = 4                     # m-tiles per kT transpose group (stage A)
RG = [list(range(N_CORES))]


def build():
    nc = bacc.Bacc("TRN2", target_bir_lowering=False, debug=False,
                   num_devices=N_CORES)

    q2 = nc.dram_tensor("q2", [BL, D], F32, kind="ExternalInput")
    k2 = nc.dram_tensor("k2", [BL, S, D], F32, kind="ExternalInput")
    v2 = nc.dram_tensor("v2", [BL, S, D], F32, kind="ExternalInput")
    Wd = nc.dram_tensor("W", [D, D], F32, kind="ExternalInput")
    Ud = nc.dram_tensor("U", [D, D], F32, kind="ExternalInput")
    out2 = nc.dram_tensor("out", [BL, S, D], F32, kind="ExternalOutput")

    k16 = nc.dram_tensor("k16", [BL, S, D], F16)
    v16 = nc.dram_tensor("v16", [BL, S, D], F16)

    # collective bounce buffers, one set per e-half
    mx_in = [nc.dram_tensor(f"mx_in{h}", [128, KC, 512], F16) for h in range(EH)]
    mx_out = [nc.dram_tensor(f"mx_out{h}", [128, KC, 512], F16) for h in range(EH)]
    sm_in = [nc.dram_tensor(f"sm_in{h}", [128, KC, 512], F16) for h in range(EH)]
    sm_out = [nc.dram_tensor(f"sm_out{h}", [128, KC, 512], F16) for h in range(EH)]

    warm_in = nc.dram_tensor("warm_in", [128, 16], F32)
    warm_out = nc.dram_tensor("warm_out", [128, 16], F32)
    warm_out2 = nc.dram_tensor("warm_out2", [128, 16], F32)

    warm_d = nc.inline_tensor(np.ones((128, 16), np.float32), name="warm_d")
    ones_d = nc.inline_tensor(np.ones((1, 128), np.float32), name="ones1")

    with tile.TileContext(nc) as tc:
        with tc.tile_pool(name="rp", bufs=1) as rp:
            # warm up the collective machinery early (first AR pays ~70us)
            wtile = rp.tile([128, 16], F32, name="wtile")
            nc.gpsimd.dma_start(wtile[:], warm_d.ap())
            nc.gpsimd.dma_start(warm_in.ap(), wtile[:])
            ar_w1 = nc.gpsimd.collective_compute(
                "AllReduce", mybir.AluOpType.max, replica_groups=RG,
                ins=[warm_in.ap().opt()], outs=[warm_out.ap().opt()])
            ar_w2 = nc.gpsimd.collective_compute(
                "AllReduce", mybir.AluOpType.add, replica_groups=RG,
                ins=[warm_out.ap().opt()], outs=[warm_out2.ap().opt()])

            # ---- t (tanh output), fp16, both batches resident ----
            tp_cm = tc.tile_pool(name="tp", bufs=1)
            tp = tp_cm.__enter__()
            t_sb = [tp.tile([128, M_T, D], F16, name=f"t{b}") for b in range(BL)]

            # ---- stage A residents: W16, ones, fac2 ----
            wp_cm = tc.tile_pool(name="wp", bufs=1)
            wp = wp_cm.__enter__()
            W16 = wp.tile([128, KC, D], F16, name="W16")
            nc.gpsimd.dma_start(W16[:], Wd.ap().rearrange("(kc p) e -> p kc e", p=128))
            ones16 = wp.tile([1, 128], F16, name="ones16")
            nc.gpsimd.dma_start(ones16[:], ones_d.ap())
            fac2 = wp.tile([1, BL, D], F16, name="fac2")

            # ---- fac2 = q @ U (f32r), also warms the PE clock ----
            with (
                tc.tile_pool(name="f2", bufs=2) as f2p,
                tc.tile_pool(name="f2u", bufs=1) as f2u,
                tc.tile_pool(name="f2ps", bufs=2, space="PSUM") as f2ps,
            ):
                U_r = f2u.tile([128, KC, D], F32R, name="U_r")
                nc.gpsimd.dma_start(
                    U_r[:], Ud.ap().rearrange("(kc p) e -> p kc e", p=128))
                qcols = []
                for b in range(BL):
                    qcol = f2p.tile([128, KC], F32R, tag="qcol", name=f"qcol{b}")
                    nc.gpsimd.dma_start(
                        qcol[:], q2.ap()[b].rearrange("(kc p) -> p kc", p=128))
                    qcols.append(qcol)

                # fp16 casts of k (per 4-m-tile chunk, so stage A can
                # start early) and v, all on the gpsimd SWDGE queue.
                for b in range(BL):
                    for mg in range(0, M_T, MG):
                        nc.gpsimd.dma_start(
                            k16.ap()[b, mg * 128:(mg + MG) * 128, :],
                            k2.ap()[b, mg * 128:(mg + MG) * 128, :])
                for b in range(BL):
                    nc.gpsimd.dma_start(v16.ap()[b], v2.ap()[b])

                for b in range(BL):
                    for h in range(EH):
                        ps = f2ps.tile([1, 512], F32, tag="f2ps", name=f"f2ps{b}_{h}")
                        for kc in range(KC):
                            nc.tensor.matmul(ps[:], qcols[b][:, kc:kc + 1],
                                             U_r[:, kc, h * 512:(h + 1) * 512],
                                             start=(kc == 0), stop=(kc == KC - 1))
                        nc.scalar.copy(fac2[0:1, b, h * 512:(h + 1) * 512], ps[:])

            # ======== stage A: t = tanh(k @ W + fac2), fp16 ========
            with (
                tc.tile_pool(name="akt", bufs=2) as ktp,
                tc.tile_pool(name="aps", bufs=2, space="PSUM") as aps,
            ):
                for b in range(BL):
                    for mg in range(0, M_T, MG):
                        ktg = ktp.tile([128, KC, MG * 128], F16, tag="ktg",
                                       name=f"ktg{b}_{mg}")
                        nc.sync.dma_start(
                            ktg[:], k16.ap()[b, mg * 128:(mg + MG) * 128, :],
                            transpose=True)
                        for j in range(MG):
                            m = mg + j
                            psh = [aps.tile([128, 512], F32, tag=f"aps{h}",
                                            name=f"aps{b}_{m}_{h}")
                                   for h in range(EH)]
                            for kc in range(KC):
                                for h in range(EH):
                                    nc.tensor.matmul(
                                        psh[h][:],
                                        ktg[:, kc, j * 128:(j + 1) * 128],
                                        W16[:, kc, h * 512:(h + 1) * 512],
                                        start=(kc == 0), stop=False)
                            for h in range(EH):
                                nc.tensor.matmul(
                                    psh[h][:], ones16[:],
                                    fac2[0:1, b, h * 512:(h + 1) * 512],
                                    start=False, stop=True)
                                nc.scalar.activation(
                                    t_sb[b][:, m, h * 512:(h + 1) * 512],
                                    psh[h][:], AF.Tanh)

            wp_cm.__exit__(None, None, None)

            # ======== stages B + softmax + C, pipelined by e-half ========
            # pool open order mirrors lifetime (LIFO close): smx/pp live
            # through stage C, sp/bp/bps die at the end of the softmax
            smx_cm = tc.tile_pool(name="smx", bufs=2)
            smx = smx_cm.__enter__()
            pp_cm = tc.tile_pool(name="pp", bufs=2)
            pp = pp_cm.__enter__()
            sp_cm = tc.tile_pool(name="sp", bufs=1)
            sp = sp_cm.__enter__()

            bp_cm = tc.tile_pool(name="bp", bufs=3)
            bp = bp_cm.__enter__()
            bps_cm = tc.tile_pool(name="bps", bufs=1, space="PSUM")
            bps = bps_cm.__enter__()

            def stage_b(h):
                s_h = []
                for b in range(BL):
                    psb = [bps.tile([128, 512], F32, tag=f"pb{dt}",
                                    name=f"pb{h}_{b}_{dt}") for dt in range(KC)]
                    for m in range(M_T):
                        vslab = bp.tile([128, D], F16, tag="vslab",
                                        name=f"vslab{h}_{b}_{m}")
                        nc.sync.dma_start(
                            vslab[:], v16.ap()[b, m * 128:(m + 1) * 128, :])
                        for dt in range(KC):
                            nc.tensor.matmul(
                                psb[dt][:],
                                vslab[:, dt * 128:(dt + 1) * 128],
                                t_sb[b][:, m, h * 512:(h + 1) * 512],
                                start=(m == 0), stop=(m == M_T - 1))
                    s_b = sp.tile([128, KC, 512], F32, tag=f"s{b}",
                                  name=f"s{h}_{b}")
                    for dt in range(KC):
                        nc.vector.tensor_copy(s_b[:, dt, :], psb[dt][:])
                    s_h.append(s_b)
                return s_h

            def local_max(h, s_h):
                # local max over the 2 batches -> f16 -> DRAM
                # (stores ride the scalar HWDGE queue: tanh is done and C
                # out-stores come much later, so nothing is blocked)
                for c in range(ARC):
                    dsl = slice(2 * c, 2 * c + 2)
                    mx = smx.tile([128, 2, 512], F16, tag="sfb", name=f"mx{h}_{c}")
                    nc.vector.tensor_max(mx[:], s_h[0][:, dsl, :],
                                         s_h[1][:, dsl, :])
                    nc.scalar.dma_start(mx_in[h].ap()[:, dsl, :], mx[:])

            def exp_and_sum(h, s_h, p_h, ew):
                # after AR-max(h): subtract gmax, exp -> p f16, local sum.
                # `ew` = engine for the pre-sum elementwise work (vector
                # for h0; gpsimd for h1 so the vector queue stays clear
                # for C(h0) PSUM evictions). exp is always scalar.
                for c in range(ARC):
                    dsl = slice(2 * c, 2 * c + 2)
                    gmxb = smx.tile([128, 2, 512], F16, tag="sfb",
                                    name=f"gmxb{h}_{c}")
                    nc.gpsimd.dma_start(gmxb[:], mx_out[h].ap()[:, dsl, :])
                    gmx = smx.tile([128, 2, 512], F32, tag="sff",
                                   name=f"gmx{h}_{c}")
                    ew.tensor_copy(gmx[:], gmxb[:])
                    for b in range(BL):
                        ew.tensor_sub(s_h[b][:, dsl, :],
                                      s_h[b][:, dsl, :], gmx[:])
                        nc.scalar.activation(p_h[b][:, dsl, :],
                                             s_h[b][:, dsl, :], AF.Exp)
                    sm = smx.tile([128, 2, 512], F16, tag="sfb", name=f"sm{h}_{c}")
                    ew.tensor_add(sm[:], p_h[0][:, dsl, :],
                                  p_h[1][:, dsl, :])
                    nc.scalar.dma_start(sm_in[h].ap()[:, dsl, :], sm[:])

            def recip_z(h):
                # after AR-sum(h): recb = 1/Z via DVE reciprocal
                recbs = []
                for c in range(ARC):
                    dsl = slice(2 * c, 2 * c + 2)
                    zz = smx.tile([128, 2, 512], F16, tag="sfb", name=f"zz{h}_{c}")
                    nc.gpsimd.dma_start(zz[:], sm_out[h].ap()[:, dsl, :])
                    rec = smx.tile([128, 2, 512], F32, tag="sff",
                                   name=f"rec{h}_{c}")
                    nc.vector.reciprocal(rec[:], zz[:])
                    recb = smx.tile([128, 2, 512], F16, tag="recb",
                                    name=f"recb{h}_{c}")
                    nc.vector.tensor_copy(recb[:], rec[:])
                    recbs.append(recb)
                return recbs

            def attn_mul(h, p_h, recbs, b):
                # attn = p * (1/Z), in place, for one batch
                for c in range(ARC):
                    dsl = slice(2 * c, 2 * c + 2)
                    nc.vector.tensor_mul(p_h[b][:, dsl, :],
                                         p_h[b][:, dsl, :], recbs[c][:])

            # p tiles (exp outputs), f16
            p_t = {}
            for h in range(EH):
                p_t[h] = [pp.tile([128, KC, 512], F16, tag=f"p{b}",
                                  name=f"p{h}_{b}") for b in range(BL)]

            s0 = stage_b(0)
            local_max(0, s0)
            ar_mx0 = nc.gpsimd.collective_compute(
                "AllReduce", mybir.AluOpType.max, replica_groups=RG,
                ins=[mx_in[0].ap().opt()], outs=[mx_out[0].ap().opt()])
            tile.add_dep_helper(ar_mx0.ins, ar_w2.ins, sync=False,
                                reason="serialize collectives")

            # h0 softmax enqueued BEFORE stage_b(1): its vector ops only
            # wait on AR-max(h0), which lands while B(h1) computes, so
            # B(h1)'s evictions (behind them in the vector queue) are
            # never actually delayed.
            exp_and_sum(0, s0, p_t[0], nc.vector)
            ar_sm0 = nc.gpsimd.collective_compute(
                "AllReduce", mybir.AluOpType.add, replica_groups=RG,
                ins=[sm_in[0].ap().opt()], outs=[sm_out[0].ap().opt()])
            tile.add_dep_helper(ar_sm0.ins, ar_mx0.ins, sync=False,
                                reason="serialize collectives")

            s1 = stage_b(1)  # PE busy while AR-max/sum(h0) fly
            local_max(1, s1)
            ar_mx1 = nc.gpsimd.collective_compute(
                "AllReduce", mybir.AluOpType.max, replica_groups=RG,
                ins=[mx_in[1].ap().opt()], outs=[mx_out[1].ap().opt()])
            tile.add_dep_helper(ar_mx1.ins, ar_sm0.ins, sync=False,
                                reason="serialize collectives")

            recb0 = recip_z(0)
            exp_and_sum(1, s1, p_t[1], nc.gpsimd)
            ar_sm1 = nc.gpsimd.collective_compute(
                "AllReduce", mybir.AluOpType.add, replica_groups=RG,
                ins=[sm_in[1].ap().opt()], outs=[sm_out[1].ap().opt()])
            tile.add_dep_helper(ar_sm1.ins, ar_mx1.ins, sync=False,
                                reason="serialize collectives")
            attn_mul(0, p_t[0], recb0, 0)
            attn_mul(0, p_t[0], recb0, 1)

            bps_cm.__exit__(None, None, None)
            bp_cm.__exit__(None, None, None)
            sp_cm.__exit__(None, None, None)

            cp_cm = tc.tile_pool(name="cp", bufs=3)
            cp = cp_cm.__enter__()
            op_cm = tc.tile_pool(name="op", bufs=12)
            op = op_cm.__enter__()
            cps_cm = tc.tile_pool(name="cps", bufs=1, space="PSUM")
            cps = cps_cm.__enter__()

            def stage_c_round(h, attn_h, b, mg):
                # kc-outer / m-inner over 8 persistent PSUM banks. vT
                # tiles arrive via one xbar-transpose DMA per round.
                vtg = cp.tile([128, KC, 1024], F16, tag="vtg",
                              name=f"vtg{h}_{b}_{mg}")
                nc.sync.dma_start(
                    vtg[:], v16.ap()[b, mg * 128:(mg + 8) * 128, :],
                    transpose=True)
                pss = [cps.tile([128, 512], F32, tag=f"cps{j}",
                                name=f"cps{h}_{b}_{mg}_{j}")
                       for j in range(8)]
                for kc in range(KC):
                    for j in range(8):
                        nc.tensor.matmul(
                            pss[j][:], vtg[:, kc, j * 128:(j + 1) * 128],
                            attn_h[b][:, kc, :],
                            start=(kc == 0), stop=(kc == KC - 1))
                for j in range(8):
                    m = mg + j
                    ost = op.tile([128, 512], F32, tag="ost",
                                  name=f"ost{h}_{b}_{m}")
                    nc.vector.tensor_copy(ost[:], pss[j][:])
                    nc.scalar.dma_start(
                        out2.ap()[b, m * 128:(m + 1) * 128,
                                  h * 512:(h + 1) * 512], ost[:])

            stage_c_round(0, p_t[0], 0, 0)
            stage_c_round(0, p_t[0], 0, 8)
            stage_c_round(0, p_t[0], 1, 0)
            # h1 1/Z lands mid-C(h0); b0's attn_mul goes first so
            # C(h1,b0) can start the moment C(h0) drains.
            recb1 = recip_z(1)
            attn_mul(1, p_t[1], recb1, 0)
            stage_c_round(0, p_t[0], 1, 8)
            attn_mul(1, p_t[1], recb1, 1)
            stage_c_round(1, p_t[1], 0, 0)
            stage_c_round(1, p_t[1], 0, 8)
            stage_c_round(1, p_t[1], 1, 0)
            stage_c_round(1, p_t[1], 1, 8)

            cps_cm.__exit__(None, None, None)
            op_cm.__exit__(None, None, None)
            cp_cm.__exit__(None, None, None)
            pp_cm.__exit__(None, None, None)
            smx_cm.__exit__(None, None, None)
            tp_cm.__exit__(None, None, None)

    nc.compile()
    return nc


_NC = None


def _get_nc():
    global _NC
    if _NC is None:
        _NC = build()
    return _NC


def kernel(q, k, v, W, U):
    q = np.ascontiguousarray(np.asarray(q, dtype=np.float32))
    k = np.ascontiguousarray(np.asarray(k, dtype=np.float32))
    v = np.ascontiguousarray(np.asarray(v, dtype=np.float32))
    W = np.ascontiguousarray(np.asarray(W, dtype=np.float32))
    U = np.ascontiguousarray(np.asarray(U, dtype=np.float32))

    nc = _get_nc()
    in_maps = [
        {
            "q2": q[c * BL:(c + 1) * BL],
            "k2": k[c * BL:(c + 1) * BL],
            "v2": v[c * BL:(c + 1) * BL],
            "W": W,
            "U": U,
        }
        for c in range(N_CORES)
    ]
    res = run_bass_kernel_spmd(nc, in_maps, core_ids=list(range(N_CORES)))
    out = np.concatenate([res.results[c]["out"] for c in range(N_CORES)], axis=0)
    return out.astype(np.float32)


if __name__ == "__main__":
    rng = np.random.default_rng(0)
    q = rng.standard_normal((B, D), dtype=np.float32)
    k = rng.standard_normal((B, S, D), dtype=np.float32)
    v = rng.standard_normal((B, S, D), dtype=np.float32)
    W = (rng.standard_normal((D, D), dtype=np.float32) / np.sqrt(D)).astype(np.float32)
    U = (rng.standard_normal((D, D), dtype=np.float32) / np.sqrt(D)).astype(np.float32)
    out = kernel(q=q, k=k, v=v, W=W, U=U)
    print("out", out.shape, out.dtype, float(np.abs(out).mean()))


# revision 10
# speedup vs baseline: 1.0724x; 1.0312x over previous
"""Distributed Trainium2 kernel for nn_Attention_31104153157828.

Computation (B=16, S=2048, D=1024):
    fac1 = k @ W                     [B,S,D]
    fac2 = (q @ U)[:, None, :]       [B,1,D]
    t    = tanh(fac1 + fac2)
    s    = einsum('bsd,bse->bde', v, t)      [B,D,D]
    attn = softmax(s, axis=0)                 (softmax over BATCH)
    out  = einsum('bsd,bde->bse', v, attn)   [B,S,D]

Sharding: data-parallel over batch, 2 batches per core on 8 cores.
The batch-axis softmax needs a cross-core AllReduce of max and sum(exp)
over the [D,D] logit matrix, pipelined by e-half so the AllReduces hide
under tensor-engine work.

v3 design:
  - All matmuls fp16; PSUM/logits f32. k cast to DRAM fp16, kT and vT
    tiles via HWDGE xbar-transpose loads (no PE transposes).
  - fac2 bias folded in via one DVE add on the PSUM tile per (m,h)
    instead of 64 rank-1 matmuls.
  - In-order queue discipline (no AR-gated op ahead of critical work):
      PE:     fac2 MMs, A MMs, B MMs, C MMs
      Scalar: tanh, exp, Ln/Exp(-x) 1/Z, mx/sm bounce STORES
      Vector: fac2 bcast evict, A bias adds, B/C PSUM evicts, softmax
              max/sub/add/mul (interleaved so C evicts never starve)
      GpSimd: casts, warm+real AR triggers, AR-gated bounce LOADS
      Sync:   kT/vT xbar transposes, v slabs, C out-stores
  - Warmup collective split: AR_w1 early, AR_w2 after the casts, so the
    gpsimd queue never blocks the k16/v16/W16 loads.
"""
import numpy as np
import concourse.bass as bass
import concourse.bacc as bacc
import concourse.tile as tile
import concourse.mybir as mybir
from concourse.bass_utils import run_bass_kernel_spmd

F32 = mybir.dt.float32
F32R = mybir.dt.float32r
F16 = mybir.dt.float16
AF = mybir.ActivationFunctionType

B, S, D = 16, 2048, 1024
N_CORES = 8
BL = B // N_CORES          # local batches per core = 2
M_T = S // 128             # 16 s-tiles
KC = D // 128              # 8 contraction chunks (d)
EH = 2                     # e halves of 512
ARC = 4                    # softmax chunks (pairs of d-tiles)
MG = 4                     # m-tiles per kT transpose group (stage A)
RG = [list(range(N_CORES))]


def build():
    nc = bacc.Bacc("TRN2", target_bir_lowering=False, debug=False,
                   num_devices=N_CORES)

    q2 = nc.dram_tensor("q2", [BL, D], F32, kind="ExternalInput")
    k2 = nc.dram_tensor("k2", [BL, S, D], F32, kind="ExternalInput")
    v2 = nc.dram_tensor("v2", [BL, S, D], F32, kind="ExternalInput")
    Wd = nc.dram_tensor("W", [D, D], F32, kind="ExternalInput")
    Ud = nc.dram_tensor("U", [D, D], F32, kind="ExternalInput")
    out2 = nc.dram_tensor("out", [BL, S, D], F32, kind="ExternalOutput")

    k16 = nc.dram_tensor("k16", [BL, S, D], F16)
    v16 = nc.dram_tensor("v16", [BL, S, D], F16)

    # collective bounce buffers, one set per e-half
    mx_in = [nc.dram_tensor(f"mx_in{h}", [128, KC, 512], F16) for h in range(EH)]
    mx_out = [nc.dram_tensor(f"mx_out{h}", [128, KC, 512], F16) for h in range(EH)]
    sm_in = [nc.dram_tensor(f"sm_in{h}", [128, KC, 512], F16) for h in range(EH)]
    sm_out = [nc.dram_tensor(f"sm_out{h}", [128, KC, 512], F16) for h in range(EH)]

    warm_in = nc.dram_tensor("warm_in", [128, 16], F32)
    warm_out = nc.dram_tensor("warm_out", [128, 16], F32)
    warm_out2 = nc.dram_tensor("warm_out2", [128, 16], F32)

    warm_d = nc.inline_tensor(np.ones((128, 16), np.float32), name="warm_d")
    ones_d = nc.inline_tensor(np.ones((1, 128), np.float32), name="ones1")

    with tile.TileContext(nc) as tc:
        with tc.tile_pool(name="rp", bufs=1) as rp:
            # ---- long-lived pools first (LIFO close order) ----
            pp_cm = tc.tile_pool(name="pp", bufs=2)
            pp = pp_cm.__enter__()
            cp_cm = tc.tile_pool(name="cp", bufs=3)
            cp = cp_cm.__enter__()
            smx_cm = tc.tile_pool(name="smx", bufs=2)
            smx = smx_cm.__enter__()
            rbp_cm = tc.tile_pool(name="rbp", bufs=4)
            rbp = rbp_cm.__enter__()
            tp_cm = tc.tile_pool(name="tp", bufs=1)
            tp = tp_cm.__enter__()
            t_sb = [tp.tile([128, M_T, D], F16, name=f"t{b}") for b in range(BL)]

            wp_cm = tc.tile_pool(name="wp", bufs=1)
            wp = wp_cm.__enter__()

            # warmup collective part 1 (fires while casts stream)
            wtile = rp.tile([128, 16], F32, name="wtile")
            nc.gpsimd.dma_start(wtile[:], warm_d.ap())
            nc.gpsimd.dma_start(warm_in.ap(), wtile[:])

            W16 = wp.tile([128, KC, D], F16, name="W16")
            nc.gpsimd.dma_start(W16[:], Wd.ap().rearrange("(kc p) e -> p kc e", p=128))
            # first half of k casts so stage A can start ASAP
            for mg in range(0, M_T, MG):
                nc.gpsimd.dma_start(
                    k16.ap()[0, mg * 128:(mg + MG) * 128, :],
                    k2.ap()[0, mg * 128:(mg + MG) * 128, :])
            fac2b = wp.tile([128, BL, D], F32, name="fac2b")

            # ---- fac2 = q @ U (f32r), broadcast to 128 partitions ----
            with (
                tc.tile_pool(name="f2", bufs=1) as f2p,
                tc.tile_pool(name="f2u", bufs=1) as f2u,
                tc.tile_pool(name="f2ps", bufs=2, space="PSUM") as f2ps,
            ):
                ones16 = f2p.tile([1, 128], F16, tag="on", name="ones16")
                nc.gpsimd.dma_start(ones16[:], ones_d.ap())
                fac2r = f2p.tile([1, BL, D], F16, tag="fr", name="fac2r")
                U_r = f2u.tile([128, KC, D], F16, name="U_r")
                nc.gpsimd.dma_start(
                    U_r[:], Ud.ap().rearrange("(kc p) e -> p kc e", p=128))
                qcols = []
                for b in range(BL):
                    qcol = f2p.tile([128, KC], F16, tag="qcol", name=f"qcol{b}")
                    nc.gpsimd.dma_start(
                        qcol[:], q2.ap()[b].rearrange("(kc p) -> p kc", p=128))
                    qcols.append(qcol)

                ar_w1 = nc.gpsimd.collective_compute(
                    "AllReduce", mybir.AluOpType.max, replica_groups=RG,
                    ins=[warm_in.ap().opt()], outs=[warm_out.ap().opt()])
                # rest of the casts; AR_w2 last so its wait blocks nothing
                for mg in range(0, M_T, MG):
                    nc.gpsimd.dma_start(
                        k16.ap()[1, mg * 128:(mg + MG) * 128, :],
                        k2.ap()[1, mg * 128:(mg + MG) * 128, :])
                for b in range(BL):
                    nc.gpsimd.dma_start(v16.ap()[b], v2.ap()[b])
                ar_w2 = nc.gpsimd.collective_compute(
                    "AllReduce", mybir.AluOpType.add, replica_groups=RG,
                    ins=[warm_out.ap().opt()], outs=[warm_out2.ap().opt()])

                for b in range(BL):
                    for h in range(EH):
                        ps = f2ps.tile([1, 512], F32, tag="f2ps",
                                       name=f"f2ps{b}_{h}")
                        for kc in range(KC):
                            nc.tensor.matmul(ps[:], qcols[b][:, kc:kc + 1],
                                             U_r[:, kc, h * 512:(h + 1) * 512],
                                             start=(kc == 0), stop=(kc == KC - 1))
                        nc.scalar.copy(fac2r[0:1, b, h * 512:(h + 1) * 512], ps[:])
                # broadcast fac2 along partitions: ones^T @ fac2r
                for b in range(BL):
                    for h in range(EH):
                        ps = f2ps.tile([128, 512], F32, tag="f2bc",
                                       name=f"f2bc{b}_{h}")
                        nc.tensor.matmul(ps[:], ones16[:],
                                         fac2r[0:1, b, h * 512:(h + 1) * 512],
                                         start=True, stop=True)
                        nc.vector.tensor_copy(fac2b[:, b, h * 512:(h + 1) * 512],
                                              ps[:])

            # ======== stage A: t = tanh(k @ W + fac2), fp16 ========
            with (
                tc.tile_pool(name="akt", bufs=2) as ktp,
                tc.tile_pool(name="aps", bufs=2, space="PSUM") as aps,
            ):
                for b in range(BL):
                    for mg in range(0, M_T, MG):
                        ktg = ktp.tile([128, KC, MG * 128], F16, tag="ktg",
                                       name=f"ktg{b}_{mg}")
                        nc.sync.dma_start(
                            ktg[:], k16.ap()[b, mg * 128:(mg + MG) * 128, :],
                            transpose=True)
                        for j in range(MG):
                            m = mg + j
                            psh = [aps.tile([128, 512], F32, tag=f"aps{h}",
                                            name=f"aps{b}_{m}_{h}")
                                   for h in range(EH)]
                            for kc in range(KC):
                                for h in range(EH):
                                    nc.tensor.matmul(
                                        psh[h][:],
                                        ktg[:, kc, j * 128:(j + 1) * 128],
                                        W16[:, kc, h * 512:(h + 1) * 512],
                                        start=(kc == 0), stop=(kc == KC - 1))
                            for h in range(EH):
                                nc.vector.tensor_add(
                                    psh[h][:], psh[h][:],
                                    fac2b[:, b, h * 512:(h + 1) * 512])
                                nc.scalar.activation(
                                    t_sb[b][:, m, h * 512:(h + 1) * 512],
                                    psh[h][:], AF.Tanh)

            wp_cm.__exit__(None, None, None)

            # ======== stages B + softmax + C, pipelined by e-half ========
            sp_cm = tc.tile_pool(name="sp", bufs=1)
            sp = sp_cm.__enter__()
            bp_cm = tc.tile_pool(name="bp", bufs=6)
            bp = bp_cm.__enter__()
            bps_cm = tc.tile_pool(name="bps", bufs=1, space="PSUM")
            bps = bps_cm.__enter__()

            def stage_b_batch(h, b):
                psb = [bps.tile([128, 512], F32, tag=f"pb{dt}",
                                name=f"pb{h}_{b}_{dt}") for dt in range(KC)]
                for m in range(M_T):
                    vslab = bp.tile([128, D], F16, tag="vslab",
                                    name=f"vslab{h}_{b}_{m}")
                    nc.sync.dma_start(
                        vslab[:], v16.ap()[b, m * 128:(m + 1) * 128, :])
                    for dt in range(KC):
                        nc.tensor.matmul(
                            psb[dt][:],
                            vslab[:, dt * 128:(dt + 1) * 128],
                            t_sb[b][:, m, h * 512:(h + 1) * 512],
                            start=(m == 0), stop=(m == M_T - 1))
                s_b = sp.tile([128, KC, 512], F16, tag=f"s{b}", name=f"s{h}_{b}")
                for dt in range(KC):
                    nc.vector.tensor_copy(s_b[:, dt, :], psb[dt][:])
                return s_b

            def vtg_load(h, b, mg):
                vtg = cp.tile([128, KC, 1024], F16, tag="vtg",
                              name=f"vtg{h}_{b}_{mg}")
                nc.sync.dma_start(
                    vtg[:], v16.ap()[b, mg * 128:(mg + 8) * 128, :],
                    transpose=True)
                return vtg

            def local_max(h, s_h):
                for c in range(ARC):
                    dsl = slice(2 * c, 2 * c + 2)
                    mx = smx.tile([128, 2, 512], F16, tag="sfb", name=f"mx{h}_{c}")
                    nc.vector.tensor_max(mx[:], s_h[0][:, dsl, :],
                                         s_h[1][:, dsl, :])
                    nc.scalar.dma_start(mx_in[h].ap()[:, dsl, :], mx[:])

            def exp_and_sum(h, s_h, p_h):
                # after AR-max(h): subtract gmax, exp -> p f16, local sum
                for c in range(ARC):
                    dsl = slice(2 * c, 2 * c + 2)
                    gmxb = smx.tile([128, 2, 512], F16, tag="sfb",
                                    name=f"gmxb{h}_{c}")
                    nc.gpsimd.dma_start(gmxb[:], mx_out[h].ap()[:, dsl, :])
                    for b in range(BL):
                        nc.vector.tensor_sub(s_h[b][:, dsl, :],
                                             s_h[b][:, dsl, :], gmxb[:])
                        nc.scalar.activation(p_h[b][:, dsl, :],
                                             s_h[b][:, dsl, :], AF.Exp)
                    sm = smx.tile([128, 2, 512], F16, tag="sfb", name=f"sm{h}_{c}")
                    nc.vector.tensor_add(sm[:], p_h[0][:, dsl, :],
                                         p_h[1][:, dsl, :])
                    nc.scalar.dma_start(sm_in[h].ap()[:, dsl, :], sm[:])

            def recip_z(h):
                # after AR-sum(h): recb = 1/Z = exp(-ln(Z)) on scalar
                recbs = []
                for c in range(ARC):
                    dsl = slice(2 * c, 2 * c + 2)
                    zz = smx.tile([128, 2, 512], F16, tag="sfb", name=f"zz{h}_{c}")
                    nc.gpsimd.dma_start(zz[:], sm_out[h].ap()[:, dsl, :])
                    rec = smx.tile([128, 2, 512], F16, tag="sfb",
                                   name=f"rec{h}_{c}")
                    nc.scalar.activation(rec[:], zz[:], AF.Ln)
                    recb = rbp.tile([128, 2, 512], F16, tag="recb",
                                    name=f"recb{h}_{c}")
                    nc.scalar.activation(recb[:], rec[:], AF.Exp, scale=-1.0)
                    recbs.append(recb)
                return recbs

            def attn_mul(h, p_h, recbs):
                for c in range(ARC):
                    dsl = slice(2 * c, 2 * c + 2)
                    for b in range(BL):
                        nc.vector.tensor_mul(p_h[b][:, dsl, :],
                                             p_h[b][:, dsl, :], recbs[c][:])

            # p tiles (exp outputs), f16
            p_t = {}
            for h in range(EH):
                p_t[h] = [pp.tile([128, KC, 512], F16, tag=f"p{b}",
                                  name=f"p{h}_{b}") for b in range(BL)]

            vtgs = {}
            s00 = stage_b_batch(0, 0)
            vtgs[(0, 0, 0)] = vtg_load(0, 0, 0)
            s01 = stage_b_batch(0, 1)
            vtgs[(0, 0, 8)] = vtg_load(0, 0, 8)
            s0 = [s00, s01]
            local_max(0, s0)
            ar_mx0 = nc.gpsimd.collective_compute(
                "AllReduce", mybir.AluOpType.max, replica_groups=RG,
                ins=[mx_in[0].ap().opt()], outs=[mx_out[0].ap().opt()])
            tile.add_dep_helper(ar_mx0.ins, ar_w2.ins, sync=False,
                                reason="serialize collectives")

            # h0 softmax enqueued BEFORE stage_b(1): its vector ops wait
            # only on AR-max(h0), which lands while B(h1) computes.
            exp_and_sum(0, s0, p_t[0])
            ar_sm0 = nc.gpsimd.collective_compute(
                "AllReduce", mybir.AluOpType.add, replica_groups=RG,
                ins=[sm_in[0].ap().opt()], outs=[sm_out[0].ap().opt()])
            tile.add_dep_helper(ar_sm0.ins, ar_mx0.ins, sync=False,
                                reason="serialize collectives")

            s10 = stage_b_batch(1, 0)
            vtgs[(0, 1, 0)] = vtg_load(0, 1, 0)
            s11 = stage_b_batch(1, 1)
            vtgs[(0, 1, 8)] = vtg_load(0, 1, 8)
            s1 = [s10, s11]
            local_max(1, s1)
            ar_mx1 = nc.gpsimd.collective_compute(
                "AllReduce", mybir.AluOpType.max, replica_groups=RG,
                ins=[mx_in[1].ap().opt()], outs=[mx_out[1].ap().opt()])
            tile.add_dep_helper(ar_mx1.ins, ar_sm0.ins, sync=False,
                                reason="serialize collectives")

            recb0 = recip_z(0)
            attn_mul(0, p_t[0], recb0)

            bps_cm.__exit__(None, None, None)
            bp_cm.__exit__(None, None, None)

            op_cm = tc.tile_pool(name="op", bufs=12)
            op = op_cm.__enter__()
            cps_cm = tc.tile_pool(name="cps", bufs=1, space="PSUM")
            cps = cps_cm.__enter__()

            def stage_c_round(h, b, mg):
                vtg = vtgs[(h, b, mg)]
                pss = [cps.tile([128, 512], F32, tag=f"cps{j}",
                                name=f"cps{h}_{b}_{mg}_{j}")
                       for j in range(8)]
                for kc in range(KC):
                    for j in range(8):
                        nc.tensor.matmul(
                            pss[j][:], vtg[:, kc, j * 128:(j + 1) * 128],
                            p_t[h][b][:, kc, :],
                            start=(kc == 0), stop=(kc == KC - 1))
                for j in range(8):
                    m = mg + j
                    ost = op.tile([128, 512], F32, tag="ost",
                                  name=f"ost{h}_{b}_{m}")
                    nc.vector.tensor_copy(ost[:], pss[j][:])
                    nc.sync.dma_start(
                        out2.ap()[b, m * 128:(m + 1) * 128,
                                  h * 512:(h + 1) * 512], ost[:])

            stage_c_round(0, 0, 0)
            # h1 softmax rides between C(h0) rounds; its vector ops are
            # enqueued after round 1's evictions so they never starve C.
            exp_and_sum(1, s1, p_t[1])
            ar_sm1 = nc.gpsimd.collective_compute(
                "AllReduce", mybir.AluOpType.add, replica_groups=RG,
                ins=[sm_in[1].ap().opt()], outs=[sm_out[1].ap().opt()])
            tile.add_dep_helper(ar_sm1.ins, ar_mx1.ins, sync=False,
                                reason="serialize collectives")
            vtgs[(1, 0, 0)] = vtg_load(1, 0, 0)
            stage_c_round(0, 0, 8)
            vtgs[(1, 0, 8)] = vtg_load(1, 0, 8)
            stage_c_round(0, 1, 0)
            recb1 = recip_z(1)
            vtgs[(1, 1, 0)] = vtg_load(1, 1, 0)
            stage_c_round(0, 1, 8)
            attn_mul(1, p_t[1], recb1)
            vtgs[(1, 1, 8)] = vtg_load(1, 1, 8)

            stage_c_round(1, 0, 0)
            stage_c_round(1, 0, 8)
            stage_c_round(1, 1, 0)
            stage_c_round(1, 1, 8)

            cps_cm.__exit__(None, None, None)
            op_cm.__exit__(None, None, None)
            sp_cm.__exit__(None, None, None)
            tp_cm.__exit__(None, None, None)
            rbp_cm.__exit__(None, None, None)
            smx_cm.__exit__(None, None, None)
            cp_cm.__exit__(None, None, None)
            pp_cm.__exit__(None, None, None)

    nc.compile()
    return nc


_NC = None


def _get_nc():
    global _NC
    if _NC is None:
        _NC = build()
    return _NC


def kernel(q, k, v, W, U):
    q = np.ascontiguousarray(np.asarray(q, dtype=np.float32))
    k = np.ascontiguousarray(np.asarray(k, dtype=np.float32))
    v = np.ascontiguousarray(np.asarray(v, dtype=np.float32))
    W = np.ascontiguousarray(np.asarray(W, dtype=np.float32))
    U = np.ascontiguousarray(np.asarray(U, dtype=np.float32))

    nc = _get_nc()
    in_maps = [
        {
            "q2": q[c * BL:(c + 1) * BL],
            "k2": k[c * BL:(c + 1) * BL],
            "v2": v[c * BL:(c + 1) * BL],
            "W": W,
            "U": U,
        }
        for c in range(N_CORES)
    ]
    res = run_bass_kernel_spmd(nc, in_maps, core_ids=list(range(N_CORES)))
    out = np.concatenate([res.results[c]["out"] for c in range(N_CORES)], axis=0)
    return out.astype(np.float32)


if __name__ == "__main__":
    rng = np.random.default_rng(0)
    q = rng.standard_normal((B, D), dtype=np.float32)
    k = rng.standard_normal((B, S, D), dtype=np.float32)
    v = rng.standard_normal((B, S, D), dtype=np.float32)
    W = (rng.standard_normal((D, D), dtype=np.float32) / np.sqrt(D)).astype(np.float32)
    U = (rng.standard_normal((D, D), dtype=np.float32) / np.sqrt(D)).astype(np.float32)
    out = kernel(q=q, k=k, v=v, W=W, U=U)
    print("out", out.shape, out.dtype, float(np.abs(out).mean()))


# revision 12
# speedup vs baseline: 1.1072x; 1.0325x over previous
"""Distributed Trainium2 kernel for nn_Attention_31104153157828.

Computation (B=16, S=2048, D=1024):
    fac1 = k @ W                     [B,S,D]
    fac2 = (q @ U)[:, None, :]       [B,1,D]
    t    = tanh(fac1 + fac2)
    s    = einsum('bsd,bse->bde', v, t)      [B,D,D]
    attn = softmax(s, axis=0)                 (softmax over BATCH)
    out  = einsum('bsd,bde->bse', v, attn)   [B,S,D]

Sharding: data-parallel over batch, 2 batches per core on 8 cores.
The batch-axis softmax needs a cross-core AllReduce of max and sum(exp)
over the [D,D] logit matrix, pipelined by e-half so the AllReduces hide
under tensor-engine work.

v3 design:
  - All matmuls fp16; PSUM/logits f32. k cast to DRAM fp16, kT and vT
    tiles via HWDGE xbar-transpose loads (no PE transposes).
  - fac2 bias folded in via one DVE add on the PSUM tile per (m,h)
    instead of 64 rank-1 matmuls.
  - In-order queue discipline (no AR-gated op ahead of critical work):
      PE:     fac2 MMs, A MMs, B MMs, C MMs
      Scalar: tanh, exp, Ln/Exp(-x) 1/Z, mx/sm bounce STORES
      Vector: fac2 bcast evict, A bias adds, B/C PSUM evicts, softmax
              max/sub/add/mul (interleaved so C evicts never starve)
      GpSimd: casts, warm+real AR triggers, AR-gated bounce LOADS
      Sync:   kT/vT xbar transposes, v slabs, C out-stores
  - Warmup collective split: AR_w1 early, AR_w2 after the casts, so the
    gpsimd queue never blocks the k16/v16/W16 loads.
"""
import numpy as np
import concourse.bass as bass
import concourse.bacc as bacc
import concourse.tile as tile
import concourse.mybir as mybir
from concourse.bass_utils import run_bass_kernel_spmd

F32 = mybir.dt.float32
F32R = mybir.dt.float32r
F16 = mybir.dt.float16
AF = mybir.ActivationFunctionType

B, S, D = 16, 2048, 1024
N_CORES = 8
BL = B // N_CORES          # local batches per core = 2
M_T = S // 128             # 16 s-tiles
KC = D // 128              # 8 contraction chunks (d)
EH = 2                     # e halves of 512
ARC = 4                    # softmax chunks (pairs of d-tiles)
MG = 8                     # m-tiles per kT transpose group (stage A)
CK = 4                     # m-tiles per k16 cast chunk
RG = [list(range(N_CORES))]


def build():
    nc = bacc.Bacc("TRN2", target_bir_lowering=False, debug=False,
                   num_devices=N_CORES)

    q2 = nc.dram_tensor("q2", [BL, D], F32, kind="ExternalInput")
    k2 = nc.dram_tensor("k2", [BL, S, D], F32, kind="ExternalInput")
    v2 = nc.dram_tensor("v2", [BL, S, D], F32, kind="ExternalInput")
    Wd = nc.dram_tensor("W", [D, D], F32, kind="ExternalInput")
    Ud = nc.dram_tensor("U", [D, D], F32, kind="ExternalInput")
    out2 = nc.dram_tensor("out", [BL, S, D], F32, kind="ExternalOutput")

    k16 = nc.dram_tensor("k16", [BL, S, D], F16)
    v16 = nc.dram_tensor("v16", [BL, S, D], F16)

    # collective bounce buffers, one set per e-half
    mx_in = [nc.dram_tensor(f"mx_in{h}", [128, KC, 512], F16) for h in range(EH)]
    mx_out = [nc.dram_tensor(f"mx_out{h}", [128, KC, 512], F16) for h in range(EH)]
    sm_in = [nc.dram_tensor(f"sm_in{h}", [128, KC, 512], F16) for h in range(EH)]
    sm_out = [nc.dram_tensor(f"sm_out{h}", [128, KC, 512], F16) for h in range(EH)]

    warm_in = nc.dram_tensor("warm_in", [128, 16], F32)
    warm_out = nc.dram_tensor("warm_out", [128, 16], F32)
    warm_out2 = nc.dram_tensor("warm_out2", [128, 16], F32)

    warm_d = nc.inline_tensor(np.ones((128, 16), np.float32), name="warm_d")
    ones_d = nc.inline_tensor(np.ones((1, 128), np.float32), name="ones1")

    with tile.TileContext(nc) as tc:
        with tc.tile_pool(name="rp", bufs=1) as rp:
            # ---- long-lived pools first (LIFO close order) ----
            cp_cm = tc.tile_pool(name="cp", bufs=4)
            cp = cp_cm.__enter__()
            smx_cm = tc.tile_pool(name="smx", bufs=2)
            smx = smx_cm.__enter__()
            rbp_cm = tc.tile_pool(name="rbp", bufs=4)
            rbp = rbp_cm.__enter__()
            tp_cm = tc.tile_pool(name="tp", bufs=1)
            tp = tp_cm.__enter__()
            t_sb = [tp.tile([128, M_T, D], F16, name=f"t{b}") for b in range(BL)]

            wp_cm = tc.tile_pool(name="wp", bufs=1)
            wp = wp_cm.__enter__()

            # warmup collective part 1 (fires while casts stream)
            wtile = rp.tile([128, 16], F32, name="wtile")
            nc.gpsimd.dma_start(wtile[:], warm_d.ap())
            nc.gpsimd.dma_start(warm_in.ap(), wtile[:])

            # first k casts so stage A's transposes can start ASAP
            for mg in range(0, 8, CK):
                nc.gpsimd.dma_start(
                    k16.ap()[0, mg * 128:(mg + CK) * 128, :],
                    k2.ap()[0, mg * 128:(mg + CK) * 128, :])
            W16 = wp.tile([128, KC, D], F16, name="W16")
            fac2b = wp.tile([128, BL, D], F32, name="fac2b")

            # ---- fac2 = q @ U (f32r), broadcast to 128 partitions ----
            with (
                tc.tile_pool(name="f2", bufs=1) as f2p,
                tc.tile_pool(name="f2u", bufs=1) as f2u,
                tc.tile_pool(name="f2ps", bufs=2, space="PSUM") as f2ps,
            ):
                ones16 = f2p.tile([1, 128], F16, tag="on", name="ones16")
                nc.gpsimd.dma_start(ones16[:], ones_d.ap())
                fac2r = f2p.tile([1, BL, D], F16, tag="fr", name="fac2r")
                U_r = f2u.tile([128, KC, D], F16, name="U_r")
                nc.gpsimd.dma_start(
                    U_r[:], Ud.ap().rearrange("(kc p) e -> p kc e", p=128))
                qcols = []
                for b in range(BL):
                    qcol = f2p.tile([128, KC], F16, tag="qcol", name=f"qcol{b}")
                    nc.gpsimd.dma_start(
                        qcol[:], q2.ap()[b].rearrange("(kc p) -> p kc", p=128))
                    qcols.append(qcol)
                nc.gpsimd.dma_start(
                    W16[:], Wd.ap().rearrange("(kc p) e -> p kc e", p=128))
                for mg in range(8, M_T, CK):
                    nc.gpsimd.dma_start(
                        k16.ap()[0, mg * 128:(mg + CK) * 128, :],
                        k2.ap()[0, mg * 128:(mg + CK) * 128, :])
                ar_w1 = nc.gpsimd.collective_compute(
                    "AllReduce", mybir.AluOpType.max, replica_groups=RG,
                    ins=[warm_in.ap().opt()], outs=[warm_out.ap().opt()])
                # rest of the casts; AR_w2 last so its wait blocks nothing
                for mg in range(0, M_T, CK):
                    nc.gpsimd.dma_start(
                        k16.ap()[1, mg * 128:(mg + CK) * 128, :],
                        k2.ap()[1, mg * 128:(mg + CK) * 128, :])
                for b in range(BL):
                    nc.gpsimd.dma_start(v16.ap()[b], v2.ap()[b])
                ar_w2 = nc.gpsimd.collective_compute(
                    "AllReduce", mybir.AluOpType.add, replica_groups=RG,
                    ins=[warm_out.ap().opt()], outs=[warm_out2.ap().opt()])

                for b in range(BL):
                    for h in range(EH):
                        ps = f2ps.tile([1, 512], F32, tag="f2ps",
                                       name=f"f2ps{b}_{h}")
                        for kc in range(KC):
                            nc.tensor.matmul(ps[:], qcols[b][:, kc:kc + 1],
                                             U_r[:, kc, h * 512:(h + 1) * 512],
                                             start=(kc == 0), stop=(kc == KC - 1))
                        nc.scalar.copy(fac2r[0:1, b, h * 512:(h + 1) * 512], ps[:])
                # broadcast fac2 along partitions: ones^T @ fac2r
                for b in range(BL):
                    for h in range(EH):
                        ps = f2ps.tile([128, 512], F32, tag="f2bc",
                                       name=f"f2bc{b}_{h}")
                        nc.tensor.matmul(ps[:], ones16[:],
                                         fac2r[0:1, b, h * 512:(h + 1) * 512],
                                         start=True, stop=True)
                        nc.vector.tensor_copy(fac2b[:, b, h * 512:(h + 1) * 512],
                                              ps[:])

            # ======== stage A: t = tanh(k @ W + fac2), fp16 ========
            with (
                tc.tile_pool(name="akt", bufs=2) as ktp,
                tc.tile_pool(name="aps", bufs=2, space="PSUM") as aps,
            ):
                for b in range(BL):
                    for mg in range(0, M_T, MG):
                        ktg = ktp.tile([128, KC, MG * 128], F16, tag="ktg",
                                       name=f"ktg{b}_{mg}")
                        nc.sync.dma_start(
                            ktg[:], k16.ap()[b, mg * 128:(mg + MG) * 128, :],
                            transpose=True)
                        for j in range(MG):
                            m = mg + j
                            psh = [aps.tile([128, 512], F32, tag=f"aps{h}",
                                            name=f"aps{b}_{m}_{h}")
                                   for h in range(EH)]
                            for kc in range(KC):
                                for h in range(EH):
                                    nc.tensor.matmul(
                                        psh[h][:],
                                        ktg[:, kc, j * 128:(j + 1) * 128],
                                        W16[:, kc, h * 512:(h + 1) * 512],
                                        start=(kc == 0), stop=(kc == KC - 1))
                            for h in range(EH):
                                nc.vector.tensor_add(
                                    psh[h][:], psh[h][:],
                                    fac2b[:, b, h * 512:(h + 1) * 512])
                                nc.scalar.activation(
                                    t_sb[b][:, m, h * 512:(h + 1) * 512],
                                    psh[h][:], AF.Tanh)

            wp_cm.__exit__(None, None, None)

            # ======== stages B + softmax + C, pipelined by e-half ========
            sp_cm = tc.tile_pool(name="sp", bufs=2)
            sp = sp_cm.__enter__()
            bp_cm = tc.tile_pool(name="bp", bufs=6)
            bp = bp_cm.__enter__()
            bps_cm = tc.tile_pool(name="bps", bufs=1, space="PSUM")
            bps = bps_cm.__enter__()

            def stage_b_batch(h, b):
                psb = [bps.tile([128, 512], F32, tag=f"pb{dt}",
                                name=f"pb{h}_{b}_{dt}") for dt in range(KC)]
                for m in range(M_T):
                    vslab = bp.tile([128, D], F16, tag="vslab",
                                    name=f"vslab{h}_{b}_{m}")
                    nc.sync.dma_start(
                        vslab[:], v16.ap()[b, m * 128:(m + 1) * 128, :])
                    for dt in range(KC):
                        nc.tensor.matmul(
                            psb[dt][:],
                            vslab[:, dt * 128:(dt + 1) * 128],
                            t_sb[b][:, m, h * 512:(h + 1) * 512],
                            start=(m == 0), stop=(m == M_T - 1))
                s_b = sp.tile([128, KC, 512], F16, tag=f"s{b}", name=f"s{h}_{b}")
                for dt in range(KC):
                    nc.vector.tensor_copy(s_b[:, dt, :], psb[dt][:])
                return s_b

            def vtg_load(h, b, mg):
                vtg = cp.tile([128, KC, 1024], F16, tag="vtg",
                              name=f"vtg{h}_{b}_{mg}")
                nc.scalar.dma_start(
                    vtg[:], v16.ap()[b, mg * 128:(mg + 8) * 128, :],
                    transpose=True)
                return vtg

            def local_max(h, s_h):
                for c in range(ARC):
                    dsl = slice(2 * c, 2 * c + 2)
                    mx = smx.tile([128, 2, 512], F16, tag="sfb", name=f"mx{h}_{c}")
                    nc.vector.tensor_max(mx[:], s_h[0][:, dsl, :],
                                         s_h[1][:, dsl, :])
                    nc.scalar.dma_start(mx_in[h].ap()[:, dsl, :], mx[:])

            def exp_and_sum(h, s_h):
                # after AR-max(h): subtract gmax, exp IN PLACE (p == s
                # tile, so no separate p pool), local sum
                for c in range(ARC):
                    dsl = slice(2 * c, 2 * c + 2)
                    gmxb = smx.tile([128, 2, 512], F16, tag="sfb",
                                    name=f"gmxb{h}_{c}")
                    nc.gpsimd.dma_start(gmxb[:], mx_out[h].ap()[:, dsl, :])
                    for b in range(BL):
                        nc.vector.tensor_sub(s_h[b][:, dsl, :],
                                             s_h[b][:, dsl, :], gmxb[:])
                        nc.scalar.activation(s_h[b][:, dsl, :],
                                             s_h[b][:, dsl, :], AF.Exp)
                    sm = smx.tile([128, 2, 512], F16, tag="sfb", name=f"sm{h}_{c}")
                    nc.vector.tensor_add(sm[:], s_h[0][:, dsl, :],
                                         s_h[1][:, dsl, :])
                    nc.scalar.dma_start(sm_in[h].ap()[:, dsl, :], sm[:])

            def recip_z(h):
                # after AR-sum(h): recb = 1/Z = exp(-ln(Z)) on scalar
                recbs = []
                for c in range(ARC):
                    dsl = slice(2 * c, 2 * c + 2)
                    zz = smx.tile([128, 2, 512], F16, tag="sfb", name=f"zz{h}_{c}")
                    nc.gpsimd.dma_start(zz[:], sm_out[h].ap()[:, dsl, :])
                    rec = smx.tile([128, 2, 512], F16, tag="sfb",
                                   name=f"rec{h}_{c}")
                    nc.scalar.activation(rec[:], zz[:], AF.Ln)
                    recb = rbp.tile([128, 2, 512], F16, tag="recb",
                                    name=f"recb{h}_{c}")
                    nc.scalar.activation(recb[:], rec[:], AF.Exp, scale=-1.0)
                    recbs.append(recb)
                return recbs

            def attn_mul(h, p_h, recbs):
                for c in range(ARC):
                    dsl = slice(2 * c, 2 * c + 2)
                    for b in range(BL):
                        nc.vector.tensor_mul(p_h[b][:, dsl, :],
                                             p_h[b][:, dsl, :], recbs[c][:])

            vtgs = {}
            p_t = {}
            s00 = stage_b_batch(0, 0)
            s01 = stage_b_batch(0, 1)
            s0 = [s00, s01]
            p_t[0] = s0
            local_max(0, s0)
            vtgs[(0, 0, 0)] = vtg_load(0, 0, 0)
            vtgs[(0, 0, 8)] = vtg_load(0, 0, 8)
            ar_mx0 = nc.gpsimd.collective_compute(
                "AllReduce", mybir.AluOpType.max, replica_groups=RG,
                ins=[mx_in[0].ap().opt()], outs=[mx_out[0].ap().opt()])
            tile.add_dep_helper(ar_mx0.ins, ar_w2.ins, sync=False,
                                reason="serialize collectives")

            # h0 softmax enqueued BEFORE stage_b(1): its vector ops wait
            # only on AR-max(h0), which lands while B(h1) computes.
            exp_and_sum(0, s0)
            ar_sm0 = nc.gpsimd.collective_compute(
                "AllReduce", mybir.AluOpType.add, replica_groups=RG,
                ins=[sm_in[0].ap().opt()], outs=[sm_out[0].ap().opt()])
            tile.add_dep_helper(ar_sm0.ins, ar_mx0.ins, sync=False,
                                reason="serialize collectives")

            vtgs[(0, 1, 0)] = vtg_load(0, 1, 0)
            vtgs[(0, 1, 8)] = vtg_load(0, 1, 8)
            s10 = stage_b_batch(1, 0)
            s11 = stage_b_batch(1, 1)
            s1 = [s10, s11]
            p_t[1] = s1
            local_max(1, s1)
            ar_mx1 = nc.gpsimd.collective_compute(
                "AllReduce", mybir.AluOpType.max, replica_groups=RG,
                ins=[mx_in[1].ap().opt()], outs=[mx_out[1].ap().opt()])
            tile.add_dep_helper(ar_mx1.ins, ar_sm0.ins, sync=False,
                                reason="serialize collectives")

            recb0 = recip_z(0)
            attn_mul(0, p_t[0], recb0)

            bps_cm.__exit__(None, None, None)
            bp_cm.__exit__(None, None, None)

            op_cm = tc.tile_pool(name="op", bufs=12)
            op = op_cm.__enter__()
            cps_cm = tc.tile_pool(name="cps", bufs=1, space="PSUM")
            cps = cps_cm.__enter__()

            def stage_c_round(h, b, mg):
                vtg = vtgs[(h, b, mg)]
                pss = [cps.tile([128, 512], F32, tag=f"cps{j}",
                                name=f"cps{h}_{b}_{mg}_{j}")
                       for j in range(8)]
                for kc in range(KC):
                    for j in range(8):
                        nc.tensor.matmul(
                            pss[j][:], vtg[:, kc, j * 128:(j + 1) * 128],
                            p_t[h][b][:, kc, :],
                            start=(kc == 0), stop=(kc == KC - 1))
                for j in range(8):
                    m = mg + j
                    ost = op.tile([128, 512], F32, tag="ost",
                                  name=f"ost{h}_{b}_{m}")
                    nc.vector.tensor_copy(ost[:], pss[j][:])
                    nc.sync.dma_start(
                        out2.ap()[b, m * 128:(m + 1) * 128,
                                  h * 512:(h + 1) * 512], ost[:])

            stage_c_round(0, 0, 0)
            # h1 softmax rides between C(h0) rounds; its vector ops are
            # enqueued after round 1's evictions so they never starve C.
            exp_and_sum(1, s1)
            ar_sm1 = nc.gpsimd.collective_compute(
                "AllReduce", mybir.AluOpType.add, replica_groups=RG,
                ins=[sm_in[1].ap().opt()], outs=[sm_out[1].ap().opt()])
            tile.add_dep_helper(ar_sm1.ins, ar_mx1.ins, sync=False,
                                reason="serialize collectives")
            vtgs[(1, 0, 0)] = vtg_load(1, 0, 0)
            stage_c_round(0, 0, 8)
            vtgs[(1, 0, 8)] = vtg_load(1, 0, 8)
            stage_c_round(0, 1, 0)
            recb1 = recip_z(1)
            vtgs[(1, 1, 0)] = vtg_load(1, 1, 0)
            stage_c_round(0, 1, 8)
            attn_mul(1, p_t[1], recb1)
            vtgs[(1, 1, 8)] = vtg_load(1, 1, 8)

            stage_c_round(1, 0, 0)
            stage_c_round(1, 0, 8)
            stage_c_round(1, 1, 0)
            stage_c_round(1, 1, 8)

            cps_cm.__exit__(None, None, None)
            op_cm.__exit__(None, None, None)
            sp_cm.__exit__(None, None, None)
            tp_cm.__exit__(None, None, None)
            rbp_cm.__exit__(None, None, None)
            smx_cm.__exit__(None, None, None)
            cp_cm.__exit__(None, None, None)

    nc.compile()
    return nc


_NC = None


def _get_nc():
    global _NC
    if _NC is None:
        _NC = build()
    return _NC


def kernel(q, k, v, W, U):
    q = np.ascontiguousarray(np.asarray(q, dtype=np.float32))
    k = np.ascontiguousarray(np.asarray(k, dtype=np.float32))
    v = np.ascontiguousarray(np.asarray(v, dtype=np.float32))
    W = np.ascontiguousarray(np.asarray(W, dtype=np.float32))
    U = np.ascontiguousarray(np.asarray(U, dtype=np.float32))

    nc = _get_nc()
    in_maps = [
        {
            "q2": q[c * BL:(c + 1) * BL],
            "k2": k[c * BL:(c + 1) * BL],
            "v2": v[c * BL:(c + 1) * BL],
            "W": W,
            "U": U,
        }
        for c in range(N_CORES)
    ]
    res = run_bass_kernel_spmd(nc, in_maps, core_ids=list(range(N_CORES)))
    out = np.concatenate([res.results[c]["out"] for c in range(N_CORES)], axis=0)
    return out.astype(np.float32)


if __name__ == "__main__":
    rng = np.random.default_rng(0)
    q = rng.standard_normal((B, D), dtype=np.float32)
    k = rng.standard_normal((B, S, D), dtype=np.float32)
    v = rng.standard_normal((B, S, D), dtype=np.float32)
    W = (rng.standard_normal((D, D), dtype=np.float32) / np.sqrt(D)).astype(np.float32)
    U = (rng.standard_normal((D, D), dtype=np.float32) / np.sqrt(D)).astype(np.float32)
    out = kernel(q=q, k=k, v=v, W=W, U=U)
    print("out", out.shape, out.dtype, float(np.abs(out).mean()))
